# revision 34
# baseline (speedup 1.0000x reference)
"""NormAttention (B=4, N=2048, C=1024, H=16, D=64) TRN2 Bass kernel.

Entry point: kernel(**inputs) -> np.ndarray [B, N, C].

Sharding: 8 NeuronCores = 4 batches x 2 head-groups (8 heads/core), SPMD
(one NEFF, per-core input slices). Host<->device traffic is minimized
(26 MB in / 16 MB out per call vs 208/64 MB for the f32 baseline):
every core uploads only a DISJOINT slice of the inputs in bf16 (x: its
batch's n-half; wqkv+wo fused into one tensor: a quarter of its
head-group's slice; rope tables: 1/8 of the rows) and the full per-core
operands are reassembled in HBM by on-device AllGathers (pairs for x,
quads for weights, all-8 for tables). The two per-batch partial outputs
are summed on device by two pair ReduceScatters (CCE bf16 add; the first
issued mid-kernel so it overlaps the remaining attention compute), so
each core returns only [N/2, C] bf16. Host reassembles quarter-shards +
b_proj. Internal DRAM tiles are double-buffered (bufs=2) so successive
kernel iterations pipeline: the next call's AllGathers overlap this
call's attention tail. Dispatch goes through a cached jitted shard_map
(same mechanism as run_bass_kernel_spmd's axon path) with donated output
buffers zeroed on device.

Per-core pipeline (QKV/proj matmuls bf16, attention matmuls fp32r; both
run at full PE rate):
  KV phase: K,V = x @ w; V staged [k, d]-natural augmented with a ones column
    (softmax denominator trick); K: per-head RMSNorm + RoPE (folded into 4
    host-precomputed tables) -> PE-transpose -> K^T stacks (head pairs).
  Per 512-wide q-block: Q (same norm/rope path, DVE-only rsqrt to avoid ACT
    table swaps) -> S^T = K^T.T @ Q^T with head-pair row-tiling (K=64 x2
    concurrent); exp on ACT (1024-wide over both heads' PSUM banks);
    U^T = [V|1].T @ E flash-accumulated in PSUM; row 64 = denominators ->
    reciprocal + gpsimd partition_broadcast -> normalized O^T stacks;
    next q-block's Q and previous block's out-proj interleaved into the
    attention loop to keep PE busy under the ACT-bound exp stream.
"""
import numpy as np
from contextlib import ExitStack

import concourse.bass as bass
import concourse.tile as tile
from concourse import bacc, mybir
from concourse.masks import make_identity
from concourse.bass_utils import run_bass_kernel_spmd

# ============================ custom DVE ops ============================


from concourse import dve_ops as _dvo
from concourse.dve_spec import (
    Spec, Src0, Src1, C0, C1, C2, C3, One, lower, _spill_c3_to_src1, sq,
)
from concourse.dve_uop import DveOpSpec
from concourse.dve_spec import _has_src1 as has_src1


def _register(name, spec, subdim=False):
    for op in _dvo.OPS:
        if op.name == name:
            return op
    shas = {}
    for ver in ("v3", "v4"):
        tmp = DveOpSpec(name=name, opcode=1, uops=lower(spec, ver=ver),
                        rd1_en=has_src1(spec))
        shas[ver] = tmp.sha(ver)
    op = _dvo.DveOp(name, spec, subdim=subdim, uops_sha=shas)
    _dvo.OPS.append(op)
    _dvo._SUB_OPCODE_FOR_NAME[op.name] = _dvo._CUSTOM_DVE_ROW_BASE + len(_dvo.OPS) - 1
    _dvo.CUSTOM_DVE_SPECS[op.name] = spec
    assert _dvo._SUB_OPCODE_FOR_NAME[op.name] < 0x20
    return op


def _ref_exp_poly(in0, in1, s0, s1, imm2):
    z = in0.astype(np.float32) * s0
    return 1.0 + z * (1.0 + z * (s1 + z * imm2))


_z = Src0 * C0
_poly_body = One + _z * (One + _z * (C1 + _z * C2))
EXP_POLY_ANT = _register(
    "EXP_POLY_ANT",
    Spec(body=_poly_body, reference=_ref_exp_poly),
)

_a = sq(Src0)
_b = sq(_a)
_c = sq(_b)
_d = sq(_c)
_e = sq(_d)
_f = sq(_e)
_pow_body = sq(_f)


def _ref_pow128(in0, in1, s0, s1, imm2):
    return in0.astype(np.float32) ** 128


POW128_ANT = _register("POW128_ANT", Spec(body=_pow_body, reference=_ref_pow128))


def emit_dve_exp(nc, out_f32r, in_ap, scale):
    """out = exp(in * scale) via three DVE ops (deg-3 poly at scale/256,
    ^128 custom, final square as a standard tensor_mul so the fp32r
    verifier sees a recognized rounding producer). Intermediates go through
    an f32 view of the output tile (in-place; DVE reads lead writes)."""
    import concourse.mybir as mybir
    out_f32 = out_f32r.bitcast(mybir.dt.float32)
    nc.vector._custom_dve(EXP_POLY_ANT, out=out_f32, in0=in_ap,
                          s0=scale / 256.0, s1=0.5, imm2=1.0 / 6.0)
    nc.vector._custom_dve(POW128_ANT, out=out_f32, in0=out_f32)
    nc.vector.tensor_mul(out_f32r, out_f32, out_f32)


# ---- DVE rsqrt: quadratic seed + Newton steps (avoids ACT sqrt-table swaps)
# seed fit on v in [0.18, 2.8] (rms^2 of unit-normal rows): 15% -> 3 NR -> 5e-6
RSQRT_SEED_C = (2.26098877, 1.50100425, 0.33539981)


def _ref_rsqrt_seed(in0, in1, s0, s1, imm2):
    v = in0.astype(np.float32)
    return s0 - v * (s1 - v * imm2)


RSQRT_SEED_ANT = _register(
    "RSQRT_SEED_ANT",
    Spec(body=C0 - Src0 * (C1 - Src0 * C2), reference=_ref_rsqrt_seed),
)


def _ref_rsqrt_nr(in0, in1, s0, s1, imm2):
    v = in0.astype(np.float32)
    y = in1.astype(np.float32)
    return y * (s0 - s1 * (v * y * y))


RSQRT_NR_ANT = _register(
    "RSQRT_NR_ANT",
    Spec(body=Src1 * (C0 - C1 * (Src0 * sq(Src1))), reference=_ref_rsqrt_nr),
)


def emit_dve_rsqrt(nc, rr_out, ss_in, v_tmp, y_tmp, inv_n, eps):
    """rr_out = 1/sqrt(ss*inv_n + eps), all [128, M] f32 SBUF tiles.
    v_tmp, y_tmp: scratch tiles of same shape."""
    import concourse.mybir as mybir
    ALU = mybir.AluOpType
    nc.vector.tensor_scalar(v_tmp, ss_in, inv_n, eps, ALU.mult, ALU.add)
    c0, c1, c2 = RSQRT_SEED_C
    nc.vector._custom_dve(RSQRT_SEED_ANT, out=y_tmp, in0=v_tmp,
                          s0=c0, s1=c1, imm2=c2)
    nc.vector._custom_dve(RSQRT_NR_ANT, out=rr_out, in0=v_tmp, in1=y_tmp,
                          s0=1.5, s1=0.5)
    nc.vector._custom_dve(RSQRT_NR_ANT, out=y_tmp, in0=v_tmp, in1=rr_out,
                          s0=1.5, s1=0.5)
    nc.vector._custom_dve(RSQRT_NR_ANT, out=rr_out, in0=v_tmp, in1=y_tmp,
                          s0=1.5, s1=0.5)

# ============================ kernel builder ============================

import numpy as np
from contextlib import ExitStack

import concourse.bass as bass
import concourse.tile as tile
from concourse import bacc, mybir
from concourse.masks import make_identity


F32 = mybir.dt.float32
F32R = mybir.dt.float32r
BF16 = mybir.dt.bfloat16
AF = mybir.ActivationFunctionType
ALU = mybir.AluOpType
AX = mybir.AxisListType

B, N, C, H, D = 4, 2048, 1024, 16, 64
HC = 8          # heads per core
EPS = 1e-6
NT = N // 128   # 16 n tiles
CT = C // 128   # 8 contraction tiles
ST_ = HC // 2   # 4 stacks of 2 heads
KT_ = N // 128  # 16 k tiles
NCH = 8         # x chunks of 256 n
SCALE = float(D) ** -0.5
# k-tiles whose exp runs on DVE instead of ACT (load balancing) — measured
# slower in the cost model: a DVE exp tile (3 ops) costs ~3.4us vs ~1.05us
# on ACT and delays the dependent U matmul; keep exp ACT-only.
DVE_EXP_KT = frozenset()
# replica groups: pairs share a batch (differ in head-group); quads share a
# head-group (differ in batch); OCT is all 8 cores.
PAIRS = [[0, 1], [2, 3], [4, 5], [6, 7]]
QUADS = [[0, 2, 4, 6], [1, 3, 5, 7]]
OCT = [[0, 1, 2, 3, 4, 5, 6, 7]]


def ap_with(ap, new_dims):
    return bass.AP(tensor=ap.tensor, offset=ap.offset, ap=new_dims)


def build_core_kernel(num_devices=8, rep=1, cc=True):
    """cc=True: disjoint bf16 input slices + on-device AllGather / output
    ReduceScatter. cc=False: pre-gathered inputs, full [N, C] output — used
    for single-core CoreSim timeline analysis (and as a no-collective
    fallback)."""
    nc = bacc.Bacc("TRN2", target_bir_lowering=False, debug=False,
                   num_devices=num_devices)
    if cc:
        xh_d = nc.dram_tensor("xh", [128, 8, CT, 128], BF16, kind="ExternalInput").ap()
        wwo_d = nc.dram_tensor("wwo", [128, 2 * 1536 + C], BF16,
                               kind="ExternalInput").ap()
        t_d = nc.dram_tensor("tqk", [512, 128], F32, kind="ExternalInput").ap()
        y_d = nc.dram_tensor("y", [N // 2, C], BF16, kind="ExternalOutput").ap()
    else:
        xg_d = nc.dram_tensor("xg", [2, 128, 8, CT, 128], BF16,
                              kind="ExternalInput").ap()
        wg_d = nc.dram_tensor("wg", [4, 128, 2, 1536], BF16,
                              kind="ExternalInput").ap()
        wog_d = nc.dram_tensor("wog", [4, 128, 1, C], BF16,
                               kind="ExternalInput").ap()
        tg_d = nc.dram_tensor("tg", [8, 512, 128], F32, kind="ExternalInput").ap()
        y_d = nc.dram_tensor("y", [N, C], BF16, kind="ExternalOutput").ap()

    with tile.TileContext(nc) as tc, ExitStack() as ctx:
        consts = ctx.enter_context(tc.tile_pool(name="consts", bufs=1))
        big = ctx.enter_context(tc.tile_pool(name="big", bufs=1))
        qt_p = ctx.enter_context(tc.tile_pool(name="qt", bufs=2))
        ot_p = ctx.enter_context(tc.tile_pool(name="ot", bufs=2))
        ph1 = ctx.enter_context(tc.tile_pool(name="ph1", bufs=2))
        sml = ctx.enter_context(tc.tile_pool(name="sml", bufs=2))
        ph2 = ctx.enter_context(tc.tile_pool(name="ph2", bufs=2))
        ph3 = ctx.enter_context(tc.tile_pool(name="ph3", bufs=2))
        psA = ctx.enter_context(tc.tile_pool(name="psA", bufs=2, space="PSUM"))
        psB = ctx.enter_context(tc.tile_pool(name="psB", bufs=2, space="PSUM"))
        if cc:
            # bufs=2 so rep k+1's bounce/AG tiles rotate: breaks the WAR
            # chain that would serialize the next prelude behind this rep's
            # last reads (steady-state pipelining across _body reps).
            dram = ctx.enter_context(tc.tile_pool(name="dram", bufs=2,
                                                  space="DRAM"))

        # ---- persistent ----
        wo_sb = big.tile([128, ST_, C], BF16)                    # 8KB/p
        w_all = big.tile([128, CT, 1536], BF16)                  # 24KB/p
        KT = big.tile([128, ST_, N], F32R)                       # 32KB/p
        Vg = big.tile([128, KT_, HC, D + 1], F32R)               # 32.5KB/p

        ident_f = consts.tile([128, 128], F32)
        make_identity(nc, ident_f)
        ident = consts.tile([128, 128], F32R)
        nc.vector.tensor_copy(ident, ident_f)
        ones_c = consts.tile([128, 1], F32)
        nc.vector.memset(ones_c, 1.0)
        eps_c = consts.tile([128, 1], F32)
        nc.vector.memset(eps_c, EPS)
        ones_b = ap_with(ones_c, [ones_c.ap[0], [0, KT_], [0, HC]])
        nc.vector.tensor_copy(Vg[:, :, :, D], ones_b)

        def qkv_matmuls(dst_ps, xt, nsl, wtile, col):
            for t in range(CT):
                nc.tensor.matmul(dst_ps, xt[:, t, 128 * nsl:128 * (nsl + 1)],
                                 wtile[:, t, col:col + 512],
                                 start=(t == 0), stop=(t == CT - 1))

        def norm_rope_transpose(pp, tab, dstT_col, kv_mode=True, defer=None):
            """pp: [128,512] psum of q or k for one n-subtile; writes
            transposed rope output into dstT_col(s) [128p, 128] slices.

            kv_mode: ACT-heavy variant for the KV phase (ACT idle there);
            otherwise ACT is kept exp-only (no Sqrt -> no table swaps) and
            the rope muls stay on DVE."""
            # sum of squares per head (ACT square -> DVE reduce)
            sq = sml.tile([128, 512], F32, tag="sq", bufs=1)
            nc.scalar.square(sq, pp)
            ss = sml.tile([128, HC], F32, tag="ss")
            nc.vector.tensor_reduce(ss, sq.rearrange("p (h d) -> p h d", h=HC),
                                    axis=AX.X, op=ALU.add)
            rr = sml.tile([128, HC], F32, tag="rr")
            if kv_mode:
                rms = sml.tile([128, HC], F32, tag="rms")
                nc.scalar.activation(rms, ss, AF.Sqrt, bias=eps_c[:, :],
                                     scale=1.0 / D)
                nc.vector.reciprocal(rr, rms)
            else:
                v_t = sml.tile([128, HC], F32, tag="rms")
                y_t = sml.tile([128, HC], F32, tag="yt")
                emit_dve_rsqrt(nc, rr, ss, v_t, y_t, 1.0 / D, EPS)

            if kv_mode:
                # evacuate psum via ACT so gpsimd can do the rope muls
                psb = sml.tile([128, 512], F32, tag="psb", bufs=1)
                nc.scalar.copy(psb, pp)
                src = psb
                mul_eng = nc.gpsimd
            else:
                src = pp
                mul_eng = nc.vector
            pr = src.rearrange("p (h d2 two) -> p h d2 two", h=HC, two=2)
            pe = pr[:, :, :, 0]
            po = pr[:, :, :, 1]

            def hb(col):
                sl = tab[:, col:col + 32]
                return ap_with(sl, [sl.ap[0], [0, HC], sl.ap[1]])
            cqe, sqo, cqo, sqe = hb(0), hb(32), hb(64), hb(96)
            m1 = sml.tile([128, HC, 32], F32, tag="m1", bufs=2)
            m2 = sml.tile([128, HC, 32], F32, tag="m2", bufs=2)
            m3 = sml.tile([128, HC, 32], F32, tag="m3", bufs=2)
            m4 = sml.tile([128, HC, 32], F32, tag="m4", bufs=2)
            mul_eng.tensor_mul(m1, pe, cqe)
            mul_eng.tensor_mul(m2, po, sqo)
            mul_eng.tensor_mul(m3, po, cqo)
            mul_eng.tensor_mul(m4, pe, sqe)
            pre = sml.tile([128, HC, 2, 32], F32, tag="pre", bufs=2)
            nc.vector.tensor_sub(pre[:, :, 0, :], m1, m2)
            nc.vector.tensor_add(pre[:, :, 1, :], m3, m4)
            rope = sml.tile([128, 512], F32R, tag="rope", bufs=2)
            rr_b = ap_with(rr, [rr.ap[0], rr.ap[1], [0, D]])
            nc.vector.tensor_mul(rope.rearrange("p (h d) -> p h d", h=HC),
                                 pre.rearrange("p h a b -> p h (a b)"), rr_b)
            if defer is not None:
                return (rope, dstT_col, kv_mode)
            emit_transposes(rope, dstT_col, kv_mode)

        def emit_transposes(rope, dstT_col, kv_mode):
            for s in range(ST_):
                tp = psB.tile([128, 128], F32R, tag="mix", bufs=1)
                nc.tensor.transpose(tp, rope[:, 128 * s:128 * (s + 1)], ident)
                if kv_mode and s % 2 == 0:
                    nc.scalar.copy(dstT_col(s), tp)
                else:
                    nc.vector.tensor_copy(dstT_col(s), tp)

        def _body():
            # ===== prelude: stage disjoint input slices, AllGather on device
            if cc:
                # internal DRAM: collective bounce + gathered operands
                # (allocated per rep from a bufs=2 pool -> double-buffered;
                # only the 8-core AG output may be pair-Shared HBM).
                # x is gathered in two chunks so the KV loop can start after
                # the first; prelude order = wwo (gates first matmul), x_a,
                # tables (needed at the first K-norm), x_b (needed at nt=4).
                x_bnc_a = dram.tile([128, 4, CT, 128], BF16, tag="x_bnc_a")
                x_bnc_b = dram.tile([128, 4, CT, 128], BF16, tag="x_bnc_b")
                xg_a = dram.tile([2, 128, 4, CT, 128], BF16, tag="xg_a")
                xg_b = dram.tile([2, 128, 4, CT, 128], BF16, tag="xg_b")
                w_bnc = dram.tile([128, 2 * 1536], BF16, tag="w_bnc")
                wo_bnc = dram.tile([128, C], BF16, tag="wo_bnc")
                wg_o = dram.tile([4, 128, 2 * 1536], BF16, tag="wg_o")
                wog_o = dram.tile([4, 128, C], BF16, tag="wog_o")
                t_bnc = dram.tile([512, 128], F32, tag="t_bnc")
                tg = dram.tile([8, 512, 128], F32, tag="tg",
                               addr_space="Shared")
                y_bnc = dram.tile([N, C], BF16, tag="y_bnc")
                yr_a = dram.tile([N // 4, C], BF16, tag="yr_a")
                yr_b1 = dram.tile([N // 8, C], BF16, tag="yr_b1")
                yr_b2 = dram.tile([N // 8, C], BF16, tag="yr_b2")
                nc.sync.dma_start(w_bnc[:], wwo_d[:, 0:3072])
                nc.sync.dma_start(wo_bnc[:], wwo_d[:, 3072:3072 + C])
                nc.sync.dma_start(x_bnc_a[:], xh_d[:, 0:4, :, :])
                nc.sync.dma_start(x_bnc_b[:], xh_d[:, 4:8, :, :])
                nc.sync.dma_start(t_bnc[:], t_d)
                nc.gpsimd.collective_compute(
                    "AllGather", ALU.bypass, replica_groups=QUADS,
                    ins=[w_bnc.opt()], outs=[wg_o.opt()])
                nc.gpsimd.collective_compute(
                    "AllGather", ALU.bypass, replica_groups=PAIRS,
                    ins=[x_bnc_a.opt()], outs=[xg_a.opt()])
                nc.gpsimd.collective_compute(
                    "AllGather", ALU.bypass, replica_groups=OCT,
                    ins=[t_bnc.opt()], outs=[tg.opt()])
                nc.gpsimd.collective_compute(
                    "AllGather", ALU.bypass, replica_groups=PAIRS,
                    ins=[x_bnc_b.opt()], outs=[xg_b.opt()])
                nc.gpsimd.collective_compute(
                    "AllGather", ALU.bypass, replica_groups=QUADS,
                    ins=[wo_bnc.opt()], outs=[wog_o.opt()])
                w_src = lambda r, j: wg_o[r, :, 1536 * j:1536 * (j + 1)]
                wo_src = lambda s: wog_o[s, :, :]

                def x_src(nt):
                    r, l = divmod(nt, 8)
                    return (xg_a[r, :, l, :, :] if l < 4
                            else xg_b[r, :, l - 4, :, :])
            else:
                tg, y_bnc = tg_d, y_d
                w_src = lambda r, j: wg_d[r, :, j, :]
                wo_src = lambda s: wog_d[s, :, 0, :]

                def x_src(nt):
                    return xg_d[nt // 8, :, nt % 8, :, :]
            for t in range(CT):
                r, j = divmod(t, 2)
                nc.sync.dma_start(w_all[:, t, :], w_src(r, j))

            def t_tile(dst, nt, is_k):
                off = 256 * int(is_k) + 128 * (nt % 2)
                nc.sync.dma_start(dst, tg[nt // 2, off:off + 128, :])

            def x_tile(dst, nt):
                nc.sync.dma_start(dst, x_src(nt))

            # ================= Phase KV =================
            pending_tp = None
            for nt in range(NT):
                    n0 = 128 * nt
                    nsl = 0
                    xt = ph1.tile([128, CT, 128], BF16, tag="xt", bufs=3)
                    x_tile(xt, nt)
                    tk_sb = sml.tile([128, 128], F32, tag="tk")
                    t_tile(tk_sb, nt, is_k=True)
                    vp = psA.tile([128, 1024], F32, tag="st", name="vp")[:, 0:512]
                    qkv_matmuls(vp, xt, nsl, w_all, 1024)
                    nc.scalar.copy(Vg[:, nt, :, 0:D],
                                   vp.rearrange("p (h d) -> p h d", h=HC))
                    kp = psA.tile([128, 1024], F32, tag="st", name="kp")[:, 0:512]
                    qkv_matmuls(kp, xt, nsl, w_all, 512)
                    if pending_tp is not None:
                        emit_transposes(*pending_tp)
                    pending_tp = norm_rope_transpose(
                        kp, tk_sb, (lambda n0=n0: (lambda s: KT[:, s, n0:n0 + 128]))(),
                        defer=True)

            if pending_tp is not None:
                emit_transposes(*pending_tp)
            for s in range(ST_):
                nc.sync.dma_start(wo_sb[:, s, :], wo_src(s))

            def q_subtile(qt_tile, ci, j, act_evac=False):
                """Q for n-subtile j (of 4) of q-block ci -> qt_tile[:, s, 128j:]."""
                nt = 4 * ci + j
                xtq = ph1.tile([128, CT, 128], BF16, tag="xt", name="xtq", bufs=3)
                x_tile(xtq, nt)
                tq_sb = sml.tile([128, 128], F32, tag="tk")
                t_tile(tq_sb, nt, is_k=False)
                qp = psA.tile([128, 512], F32, tag="qk", bufs=1)
                qkv_matmuls(qp, xtq, 0, w_all, 0)
                norm_rope_transpose(
                    qp, tq_sb, lambda s: qt_tile[:, s, 128 * j:128 * (j + 1)],
                    kv_mode=act_evac)

            def proj_tile(ot_tile, ci, ntl, cc, alt=False):
                """alt=True (tail only, attention done): use a psA 'st' bank
                so back-to-back projs don't serialize on the single 'mix'
                buffer's DVE evacuation."""
                nt = 4 * ci + ntl
                if alt:
                    yp = psA.tile([128, 1024], F32, tag="st",
                                  name="yp_alt")[:, 0:512]
                else:
                    yp = psB.tile([128, 512], F32, tag="mix", bufs=1)
                for s in range(ST_):
                    nc.tensor.matmul(yp, ot_tile[:, s, 128 * ntl:128 * (ntl + 1)],
                                     wo_sb[:, s, 512 * cc:512 * (cc + 1)],
                                     start=(s == 0), stop=(s == ST_ - 1))
                ysb = ph3.tile([128, 512], BF16, tag="ysb")
                nc.vector.tensor_copy(ysb, yp)
                nc.sync.dma_start(
                    y_bnc[128 * nt:128 * (nt + 1), 512 * cc:512 * (cc + 1)], ysb)

            # ================= per q-block: attn (+ next Q, prev proj) ==========
            QT = qt_p.tile([128, ST_, 512], F32R, tag="QT")
            for j in range(4):
                q_subtile(QT, 0, j, act_evac=True)
            prev = None  # (OT, ci) pending projection

            for ci in range(4):
                OT = ot_p.tile([128, ST_, 512], BF16, tag="OT")
                QT_next = None
                if ci + 1 < 4:
                    QT_next = qt_p.tile([128, ST_, 512], F32R, tag="QT")
                for hp in range(ST_):
                    u = psB.tile([D + 1, 1024], F32, tag="u", bufs=1)
                    es = []
                    for kt in range(KT_):
                        st = psA.tile([128, 1024], F32, tag="st")
                        nc.tensor.matmul(st[:, 0:512],
                                         KT[0:64, hp, 128 * kt:128 * (kt + 1)],
                                         QT[0:64, hp, :],
                                         start=True, stop=True, tile_position=(0, 0))
                        nc.tensor.matmul(st[:, 512:1024],
                                         KT[64:128, hp, 128 * kt:128 * (kt + 1)],
                                         QT[64:128, hp, :],
                                         start=True, stop=True, tile_position=(64, 0))
                        e = ph2.tile([128, 1024], F32R, tag="E", bufs=2)
                        if kt in DVE_EXP_KT:
                            emit_dve_exp(nc, e, st, SCALE)
                        else:
                            nc.scalar.activation(e, st, AF.Exp, scale=SCALE)
                        es.append((kt, e))
                        if len(es) > 2:
                            pk, pe_ = es.pop(0)
                            nc.tensor.matmul(u[:, 0:512], Vg[:, pk, 2 * hp, :],
                                             pe_[:, 0:512],
                                             start=(pk == 0), stop=False)
                            nc.tensor.matmul(u[:, 512:1024], Vg[:, pk, 2 * hp + 1, :],
                                             pe_[:, 512:1024],
                                             start=(pk == 0), stop=False)
                    while es:
                        pk, pe_ = es.pop(0)
                        nc.tensor.matmul(u[:, 0:512], Vg[:, pk, 2 * hp, :],
                                         pe_[:, 0:512],
                                         start=(pk == 0), stop=(pk == KT_ - 1))
                        nc.tensor.matmul(u[:, 512:1024], Vg[:, pk, 2 * hp + 1, :],
                                         pe_[:, 512:1024],
                                         start=(pk == 0), stop=(pk == KT_ - 1))

                    # evacuate U fast to free the PSUM bank, normalize off-path
                    usb = ph2.tile([D + 1, 1024], F32, tag="usb", bufs=1)
                    nc.vector.tensor_copy(usb, u)
                    den = ph2.tile([1, 1024], F32, tag="den", bufs=1)
                    nc.vector.tensor_copy(den, usb[D:D + 1, :])
                    rcp = ph2.tile([1, 1024], F32, tag="rcp", bufs=1)
                    nc.vector.reciprocal_approx_fast(rcp, den)
                    bc = ph2.tile([64, 1024], F32, tag="bc", bufs=1)
                    nc.gpsimd.partition_broadcast(bc, rcp)
                    for e_i in range(2):
                        nc.vector.tensor_mul(
                            OT[64 * e_i:64 * (e_i + 1), hp, :],
                            usb[0:D, 512 * e_i:512 * (e_i + 1)],
                            bc[:, 512 * e_i:512 * (e_i + 1)])

                    # interleave: one Q subtile of next block + 2 proj tiles of prev
                    if QT_next is not None:
                        q_subtile(QT_next, ci + 1, hp)
                    if prev is not None:
                        proj_tile(prev[0], prev[1], hp, 0)
                        proj_tile(prev[0], prev[1], hp, 1)

                prev = (OT, ci)
                QT = QT_next
                if cc and ci == 2:
                    # rows 0:N/2 of y_bnc are complete (ci=0 projs ran during
                    # ci=1, ci=1 projs during ci=2) -> overlap first RS with
                    # the remaining attention compute
                    nc.gpsimd.collective_compute(
                        "ReduceScatter", ALU.add, replica_groups=PAIRS,
                        ins=[y_bnc[0:N // 2, :].opt()], outs=[yr_a.opt()])
                if cc and ci == 3:
                    # rows N/2:3N/4 (ci=2's projs, interleaved into ci=3's
                    # loop) are complete -> overlap with the tail projs
                    nc.gpsimd.collective_compute(
                        "ReduceScatter", ALU.add, replica_groups=PAIRS,
                        ins=[y_bnc[N // 2:3 * N // 4, :].opt()],
                        outs=[yr_b1.opt()])

            for ntl in range(4):
                proj_tile(prev[0], prev[1], ntl, 0, alt=True)
                proj_tile(prev[0], prev[1], ntl, 1)

            # ===== tail: pair-sum the remaining partial output rows.
            # rows N/2:3N/4 (ci=2's projs) completed when the ci loop ended,
            # so their RS overlaps the final 8 proj tiles; only the last
            # N/4-row RS is a true serial tail.
            if cc:
                nc.gpsimd.collective_compute(
                    "ReduceScatter", ALU.add, replica_groups=PAIRS,
                    ins=[y_bnc[3 * N // 4:N, :].opt()], outs=[yr_b2.opt()])
                nc.sync.dma_start(y_d[0:N // 4, :], yr_a[:])
                nc.sync.dma_start(y_d[N // 4:3 * N // 8, :], yr_b1[:])
                nc.sync.dma_start(y_d[3 * N // 8:N // 2, :], yr_b2[:])

        for _rep in range(rep):
            _body()


    nc.compile()
    return nc


def make_tables(freqs_cos, freqs_sin, nw):
    """Host: fold norm weight into rope tables. [N, 128] f32:
    cols 0:32=cqe, 32:64=sqo, 64:96=cqo, 96:128=sqe."""
    cos_p = np.asarray(freqs_cos)[:, 0::2]
    sin_p = np.asarray(freqs_sin)[:, 0::2]
    nw = np.asarray(nw)
    ne = nw[0::2][None, :]
    no = nw[1::2][None, :]
    return np.concatenate([cos_p * ne, sin_p * no, cos_p * no, sin_p * ne],
                          axis=1).astype(np.float32)


def shard_inputs(x, w_qkv, w_proj, b_proj, qn_w, kn_w, freqs_cos, freqs_sin):
    """Returns in_maps for 8 cores. Core c: batch c//2, head group c%2.

    Each core gets only a DISJOINT bf16 slice; the kernel AllGathers:
      xh   [128, 8, CT, 128]  x^T for n-half (c%2) of batch c//2
      wqkv [128, 2, 1536]     ct-quarter (c//2) of head-group (c%2) cols
      wo   [128, 1, C]        row-stack (c//2) of head-group (c%2)
      tqk  [512, 128]         rows 256c..256c+255 of [tq; tk] (f32)
    """
    import ml_dtypes
    BF = ml_dtypes.bfloat16
    x = np.asarray(x); w_qkv = np.asarray(w_qkv); w_proj = np.asarray(w_proj)
    tq_t = make_tables(freqs_cos, freqs_sin, qn_w).reshape(8, 2, 128, 128)
    tk_t = make_tables(freqs_cos, freqs_sin, kn_w).reshape(8, 2, 128, 128)
    tqk = np.concatenate([tq_t, tk_t], axis=1).reshape(8, 512, 128)

    xT_b = []
    for b in range(B):
        xb = x[b].astype(BF).reshape(NT, 128, CT, 128).transpose(3, 0, 2, 1)
        xT_b.append(xb)
    w_bf = w_qkv.astype(BF)
    wg_l = []
    for g in range(2):
        cols = slice(512 * g, 512 * (g + 1))
        wq_g = np.concatenate(
            [w_bf[:, 0:C][:, cols], w_bf[:, C:2 * C][:, cols],
             w_bf[:, 2 * C:3 * C][:, cols]], axis=1)
        wg_l.append(wq_g.reshape(CT, 128, 3 * 512))
    wo_bf = w_proj.astype(BF)

    in_maps = []
    for c in range(8):
        b, g = c // 2, c % 2
        wq_in = wg_l[g][2 * b:2 * b + 2].transpose(1, 0, 2).reshape(128, 3072)
        wo_in = wo_bf[512 * g:512 * (g + 1)].reshape(ST_, 128, C)[b]
        in_maps.append({
            "xh": np.ascontiguousarray(xT_b[b][:, 8 * g:8 * (g + 1)]),
            "wwo": np.ascontiguousarray(
                np.concatenate([wq_in, wo_in], axis=1)),
            "tqk": np.ascontiguousarray(tqk[c]),
        })
    return in_maps


def gather_outputs(results, b_proj):
    """Per core, y = [yr_a (512 rows) | yr_b1 (256) | yr_b2 (256)]: the rank-r
    shard of RS(rows 0:1024), RS(rows 1024:1536), RS(rows 1536:2048)."""
    out = np.empty((B, N, C), dtype=np.float32)
    bp = np.asarray(b_proj, dtype=np.float32)
    Q, E = N // 4, N // 8
    for b in range(B):
        y0 = results[2 * b]["y"].astype(np.float32)
        y1 = results[2 * b + 1]["y"].astype(np.float32)
        out[b, 0:Q] = y0[0:Q] + bp
        out[b, Q:2 * Q] = y1[0:Q] + bp
        out[b, 2 * Q:2 * Q + E] = y0[Q:Q + E] + bp
        out[b, 2 * Q + E:3 * Q] = y1[Q:Q + E] + bp
        out[b, 3 * Q:3 * Q + E] = y0[Q + E:2 * Q] + bp
        out[b, 3 * Q + E:N] = y1[Q + E:2 * Q] + bp
    return out


_CACHED = {}


def _make_runner(nc, n_cores=8):
    """Build the jitted SPMD dispatch once (same mechanism as
    run_bass_kernel_spmd's axon path, but cached across calls, with
    donated output buffers zero-filled ON DEVICE instead of shipped from
    host). Returns (dispatch, zero_fns, in_names, out_names, out_avals,
    sharding)."""
    import jax
    from jax.sharding import Mesh, PartitionSpec, NamedSharding
    from jax.experimental.shard_map import shard_map
    from concourse import bass2jax

    bass2jax.install_neuronx_cc_hook()
    partition_name = (nc.partition_id_tensor.name
                      if nc.partition_id_tensor else None)
    in_names, out_names, out_avals = [], [], []
    for alloc in nc.m.functions[0].allocations:
        if not isinstance(alloc, mybir.MemoryLocationSet):
            continue
        name = alloc.memorylocations[0].name
        if alloc.kind == "ExternalInput":
            if name != partition_name:
                in_names.append(name)
        elif alloc.kind == "ExternalOutput":
            out_names.append(name)
            out_avals.append(jax.core.ShapedArray(
                tuple(alloc.tensor_shape), mybir.dt.np(alloc.dtype)))
    n_params, n_outs = len(in_names), len(out_avals)
    all_in = in_names + out_names + ([partition_name] if partition_name else [])

    def _body(*args):
        operands = list(args)
        if partition_name:
            operands.append(bass2jax.partition_id_tensor())
        return tuple(bass2jax._bass_exec_p.bind(
            *operands, out_avals=tuple(out_avals), in_names=tuple(all_in),
            out_names=tuple(out_names), lowering_input_output_aliases=(),
            sim_require_finite=True, sim_require_nnan=True, nc=nc))

    donate = tuple(range(n_params, n_params + n_outs))
    mesh = Mesh(np.asarray(jax.devices()[:n_cores]), ("core",))
    spec = NamedSharding(mesh, PartitionSpec("core"))
    in_specs = (PartitionSpec("core"),) * (n_params + n_outs)
    out_specs = (PartitionSpec("core"),) * n_outs
    dispatch = jax.jit(
        shard_map(_body, mesh=mesh, in_specs=in_specs, out_specs=out_specs,
                  check_rep=False),
        donate_argnums=donate, keep_unused=True)
    zero_fns = [jax.jit(
        (lambda s, d: (lambda: jax.numpy.zeros((n_cores * s[0],) + s[1:], d)))(
            tuple(a.shape), a.dtype),
        out_shardings=spec) for a in out_avals]
    return dispatch, zero_fns, in_names, out_names, out_avals, spec


def _run(runner, in_maps):
    dispatch, zero_fns, in_names, out_names, out_avals, _ = runner
    n = len(in_maps)
    concat_in = [np.concatenate([np.asarray(in_maps[c][nm]) for c in range(n)],
                                axis=0) for nm in in_names]
    zeros = [f() for f in zero_fns]
    outs = dispatch(*concat_in, *zeros)
    outs_np = [np.asarray(a) for a in outs]
    return [{nm: outs_np[i].reshape(n, *out_avals[i].shape)[c]
             for i, nm in enumerate(out_names)} for c in range(n)]


def kernel(x, w_qkv, w_proj, b_proj, qn_w, kn_w, freqs_cos, freqs_sin):
    """Full-input entry point; shards across 8 NeuronCores, returns [B,N,C]."""
    in_maps = shard_inputs(x, w_qkv, w_proj, b_proj, qn_w, kn_w,
                           freqs_cos, freqs_sin)
    if "nc" not in _CACHED:
        _CACHED["nc"] = build_core_kernel(num_devices=8)
    nc = _CACHED["nc"]
    try:
        if "runner" not in _CACHED:
            _CACHED["runner"] = _make_runner(nc, 8)
        res = _run(_CACHED["runner"], in_maps)
    except Exception:
        res = run_bass_kernel_spmd(nc, in_maps, core_ids=list(range(8))).results
    return gather_outputs(res, b_proj)



# revision 35
# speedup vs baseline: 1.0195x; 1.0195x over previous
"""NormAttention (B=4, N=2048, C=1024, H=16, D=64) TRN2 Bass kernel.

Entry point: kernel(**inputs) -> np.ndarray [B, N, C].

Sharding: 8 NeuronCores = 4 batches x 2 head-groups (8 heads/core), SPMD
(one NEFF, per-core input slices). Host<->device traffic is minimized
(26 MB in / 16 MB out per call vs 208/64 MB for the f32 baseline):
every core uploads only a DISJOINT slice of the inputs in bf16 (x: its
batch's n-half; wqkv+wo fused into one tensor: a quarter of its
head-group's slice; rope tables: 1/8 of the rows) and the full per-core
operands are reassembled in HBM by on-device AllGathers (pairs for x,
quads for weights, all-8 for tables). The two per-batch partial outputs
are summed on device by two pair ReduceScatters (CCE bf16 add; the first
issued mid-kernel so it overlaps the remaining attention compute), so
each core returns only [N/2, C] bf16. Host reassembles quarter-shards +
b_proj. Internal DRAM tiles are double-buffered (bufs=2) so successive
kernel iterations pipeline: the next call's AllGathers overlap this
call's attention tail. Dispatch goes through a cached jitted shard_map
(same mechanism as run_bass_kernel_spmd's axon path) with donated output
buffers zeroed on device.

Per-core pipeline (QKV/proj matmuls bf16, attention matmuls fp32r; both
run at full PE rate):
  KV phase: K,V = x @ w; V staged [k, d]-natural augmented with a ones column
    (softmax denominator trick); K: per-head RMSNorm + RoPE (folded into 4
    host-precomputed tables) -> PE-transpose -> K^T stacks (head pairs).
  Per 512-wide q-block: Q (same norm/rope path, DVE-only rsqrt to avoid ACT
    table swaps) -> S^T = K^T.T @ Q^T with head-pair row-tiling (K=64 x2
    concurrent); exp on ACT (1024-wide over both heads' PSUM banks);
    U^T = [V|1].T @ E flash-accumulated in PSUM; row 64 = denominators ->
    reciprocal + gpsimd partition_broadcast -> normalized O^T stacks;
    next q-block's Q and previous block's out-proj interleaved into the
    attention loop to keep PE busy under the ACT-bound exp stream.
"""
import numpy as np
from contextlib import ExitStack

import concourse.bass as bass
import concourse.tile as tile
from concourse import bacc, mybir
from concourse.masks import make_identity
from concourse.bass_utils import run_bass_kernel_spmd

# ============================ custom DVE ops ============================


from concourse import dve_ops as _dvo
from concourse.dve_spec import (
    Spec, Src0, Src1, C0, C1, C2, C3, One, lower, _spill_c3_to_src1, sq,
)
from concourse.dve_uop import DveOpSpec
from concourse.dve_spec import _has_src1 as has_src1


def _register(name, spec, subdim=False):
    for op in _dvo.OPS:
        if op.name == name:
            return op
    shas = {}
    for ver in ("v3", "v4"):
        tmp = DveOpSpec(name=name, opcode=1, uops=lower(spec, ver=ver),
                        rd1_en=has_src1(spec))
        shas[ver] = tmp.sha(ver)
    op = _dvo.DveOp(name, spec, subdim=subdim, uops_sha=shas)
    _dvo.OPS.append(op)
    _dvo._SUB_OPCODE_FOR_NAME[op.name] = _dvo._CUSTOM_DVE_ROW_BASE + len(_dvo.OPS) - 1
    _dvo.CUSTOM_DVE_SPECS[op.name] = spec
    assert _dvo._SUB_OPCODE_FOR_NAME[op.name] < 0x20
    return op


def _ref_exp_poly(in0, in1, s0, s1, imm2):
    z = in0.astype(np.float32) * s0
    return 1.0 + z * (1.0 + z * (s1 + z * imm2))


_z = Src0 * C0
_poly_body = One + _z * (One + _z * (C1 + _z * C2))
EXP_POLY_ANT = _register(
    "EXP_POLY_ANT",
    Spec(body=_poly_body, reference=_ref_exp_poly),
)

_a = sq(Src0)
_b = sq(_a)
_c = sq(_b)
_d = sq(_c)
_e = sq(_d)
_f = sq(_e)
_pow_body = sq(_f)


def _ref_pow128(in0, in1, s0, s1, imm2):
    return in0.astype(np.float32) ** 128


POW128_ANT = _register("POW128_ANT", Spec(body=_pow_body, reference=_ref_pow128))


def emit_dve_exp(nc, out_f32r, in_ap, scale):
    """out = exp(in * scale) via three DVE ops (deg-3 poly at scale/256,
    ^128 custom, final square as a standard tensor_mul so the fp32r
    verifier sees a recognized rounding producer). Intermediates go through
    an f32 view of the output tile (in-place; DVE reads lead writes)."""
    import concourse.mybir as mybir
    out_f32 = out_f32r.bitcast(mybir.dt.float32)
    nc.vector._custom_dve(EXP_POLY_ANT, out=out_f32, in0=in_ap,
                          s0=scale / 256.0, s1=0.5, imm2=1.0 / 6.0)
    nc.vector._custom_dve(POW128_ANT, out=out_f32, in0=out_f32)
    nc.vector.tensor_mul(out_f32r, out_f32, out_f32)


# ---- DVE rsqrt: quadratic seed + Newton steps (avoids ACT sqrt-table swaps)
# seed fit on v in [0.18, 2.8] (rms^2 of unit-normal rows): 15% -> 3 NR -> 5e-6
RSQRT_SEED_C = (2.26098877, 1.50100425, 0.33539981)


def _ref_rsqrt_seed(in0, in1, s0, s1, imm2):
    v = in0.astype(np.float32)
    return s0 - v * (s1 - v * imm2)


RSQRT_SEED_ANT = _register(
    "RSQRT_SEED_ANT",
    Spec(body=C0 - Src0 * (C1 - Src0 * C2), reference=_ref_rsqrt_seed),
)


def _ref_rsqrt_nr(in0, in1, s0, s1, imm2):
    v = in0.astype(np.float32)
    y = in1.astype(np.float32)
    return y * (s0 - s1 * (v * y * y))


RSQRT_NR_ANT = _register(
    "RSQRT_NR_ANT",
    Spec(body=Src1 * (C0 - C1 * (Src0 * sq(Src1))), reference=_ref_rsqrt_nr),
)


def emit_dve_rsqrt(nc, rr_out, ss_in, v_tmp, y_tmp, inv_n, eps):
    """rr_out = 1/sqrt(ss*inv_n + eps), all [128, M] f32 SBUF tiles.
    v_tmp, y_tmp: scratch tiles of same shape."""
    import concourse.mybir as mybir
    ALU = mybir.AluOpType
    nc.vector.tensor_scalar(v_tmp, ss_in, inv_n, eps, ALU.mult, ALU.add)
    c0, c1, c2 = RSQRT_SEED_C
    nc.vector._custom_dve(RSQRT_SEED_ANT, out=y_tmp, in0=v_tmp,
                          s0=c0, s1=c1, imm2=c2)
    nc.vector._custom_dve(RSQRT_NR_ANT, out=rr_out, in0=v_tmp, in1=y_tmp,
                          s0=1.5, s1=0.5)
    nc.vector._custom_dve(RSQRT_NR_ANT, out=y_tmp, in0=v_tmp, in1=rr_out,
                          s0=1.5, s1=0.5)
    nc.vector._custom_dve(RSQRT_NR_ANT, out=rr_out, in0=v_tmp, in1=y_tmp,
                          s0=1.5, s1=0.5)

# ============================ kernel builder ============================

import numpy as np
from contextlib import ExitStack

import concourse.bass as bass
import concourse.tile as tile
from concourse import bacc, mybir
from concourse.masks import make_identity


F32 = mybir.dt.float32
F32R = mybir.dt.float32r
BF16 = mybir.dt.bfloat16
AF = mybir.ActivationFunctionType
ALU = mybir.AluOpType
AX = mybir.AxisListType

B, N, C, H, D = 4, 2048, 1024, 16, 64
HC = 8          # heads per core
EPS = 1e-6
NT = N // 128   # 16 n tiles
CT = C // 128   # 8 contraction tiles
ST_ = HC // 2   # 4 stacks of 2 heads
KT_ = N // 128  # 16 k tiles
NCH = 8         # x chunks of 256 n
SCALE = float(D) ** -0.5
# k-tiles whose exp runs on DVE instead of ACT (load balancing) — measured
# slower in the cost model: a DVE exp tile (3 ops) costs ~3.4us vs ~1.05us
# on ACT and delays the dependent U matmul; keep exp ACT-only.
DVE_EXP_KT = frozenset()
# replica groups: pairs share a batch (differ in head-group); quads share a
# head-group (differ in batch); OCT is all 8 cores.
PAIRS = [[0, 1], [2, 3], [4, 5], [6, 7]]
QUADS = [[0, 2, 4, 6], [1, 3, 5, 7]]
OCT = [[0, 1, 2, 3, 4, 5, 6, 7]]


def ap_with(ap, new_dims):
    return bass.AP(tensor=ap.tensor, offset=ap.offset, ap=new_dims)


def build_core_kernel(num_devices=8, rep=1, cc=True):
    """cc=True: disjoint bf16 input slices + on-device AllGather / output
    ReduceScatter. cc=False: pre-gathered inputs, full [N, C] output — used
    for single-core CoreSim timeline analysis (and as a no-collective
    fallback)."""
    nc = bacc.Bacc("TRN2", target_bir_lowering=False, debug=False,
                   num_devices=num_devices)
    if cc:
        xh_d = nc.dram_tensor("xh", [128, 8, CT, 128], BF16, kind="ExternalInput").ap()
        wwo_d = nc.dram_tensor("wwo", [128, 2 * 1536 + C], BF16,
                               kind="ExternalInput").ap()
        t_d = nc.dram_tensor("tqk", [512, 128], F32, kind="ExternalInput").ap()
        y_d = nc.dram_tensor("y", [N // 2, C], BF16, kind="ExternalOutput").ap()
    else:
        xg_d = nc.dram_tensor("xg", [2, 128, 8, CT, 128], BF16,
                              kind="ExternalInput").ap()
        wg_d = nc.dram_tensor("wg", [4, 128, 2, 1536], BF16,
                              kind="ExternalInput").ap()
        wog_d = nc.dram_tensor("wog", [4, 128, 1, C], BF16,
                               kind="ExternalInput").ap()
        tg_d = nc.dram_tensor("tg", [8, 512, 128], F32, kind="ExternalInput").ap()
        y_d = nc.dram_tensor("y", [N, C], BF16, kind="ExternalOutput").ap()

    with tile.TileContext(nc) as tc, ExitStack() as ctx:
        consts = ctx.enter_context(tc.tile_pool(name="consts", bufs=1))
        big = ctx.enter_context(tc.tile_pool(name="big", bufs=1))
        qt_p = ctx.enter_context(tc.tile_pool(name="qt", bufs=2))
        ot_p = ctx.enter_context(tc.tile_pool(name="ot", bufs=2))
        ph1 = ctx.enter_context(tc.tile_pool(name="ph1", bufs=2))
        sml = ctx.enter_context(tc.tile_pool(name="sml", bufs=2))
        ph2 = ctx.enter_context(tc.tile_pool(name="ph2", bufs=2))
        ph3 = ctx.enter_context(tc.tile_pool(name="ph3", bufs=2))
        psA = ctx.enter_context(tc.tile_pool(name="psA", bufs=2, space="PSUM"))
        psB = ctx.enter_context(tc.tile_pool(name="psB", bufs=2, space="PSUM"))
        if cc:
            # bufs=2 so rep k+1's bounce/AG tiles rotate: breaks the WAR
            # chain that would serialize the next prelude behind this rep's
            # last reads (steady-state pipelining across _body reps).
            dram = ctx.enter_context(tc.tile_pool(name="dram", bufs=2,
                                                  space="DRAM"))

        # ---- persistent ----
        wo_sb = big.tile([128, ST_, C], BF16)                    # 8KB/p
        w_all = big.tile([128, CT, 1536], BF16)                  # 24KB/p
        KT = big.tile([128, ST_, N], F32R)                       # 32KB/p
        Vg = big.tile([128, KT_, HC, D + 1], F32R)               # 32.5KB/p

        ident_f = consts.tile([128, 128], F32)
        make_identity(nc, ident_f)
        ident = consts.tile([128, 128], F32R)
        nc.vector.tensor_copy(ident, ident_f)
        ones_c = consts.tile([128, 1], F32)
        nc.vector.memset(ones_c, 1.0)
        eps_c = consts.tile([128, 1], F32)
        nc.vector.memset(eps_c, EPS)
        ones_b = ap_with(ones_c, [ones_c.ap[0], [0, KT_], [0, HC]])
        nc.vector.tensor_copy(Vg[:, :, :, D], ones_b)

        def qkv_matmuls(dst_ps, xt, nsl, wtile, col):
            for t in range(CT):
                nc.tensor.matmul(dst_ps, xt[:, t, 128 * nsl:128 * (nsl + 1)],
                                 wtile[:, t, col:col + 512],
                                 start=(t == 0), stop=(t == CT - 1))

        def norm_rope_transpose(pp, tab, dstT_col, kv_mode=True, defer=None):
            """pp: [128,512] psum of q or k for one n-subtile; writes
            transposed rope output into dstT_col(s) [128p, 128] slices.

            kv_mode: ACT-heavy variant for the KV phase (ACT idle there);
            otherwise ACT is kept exp-only (no Sqrt -> no table swaps) and
            the rope muls stay on DVE."""
            # sum of squares per head (ACT square -> DVE reduce)
            sq = sml.tile([128, 512], F32, tag="sq", bufs=1)
            nc.scalar.square(sq, pp)
            ss = sml.tile([128, HC], F32, tag="ss")
            nc.vector.tensor_reduce(ss, sq.rearrange("p (h d) -> p h d", h=HC),
                                    axis=AX.X, op=ALU.add)
            rr = sml.tile([128, HC], F32, tag="rr")
            if kv_mode:
                rms = sml.tile([128, HC], F32, tag="rms")
                nc.scalar.activation(rms, ss, AF.Sqrt, bias=eps_c[:, :],
                                     scale=1.0 / D)
                nc.vector.reciprocal(rr, rms)
            else:
                v_t = sml.tile([128, HC], F32, tag="rms")
                y_t = sml.tile([128, HC], F32, tag="yt")
                emit_dve_rsqrt(nc, rr, ss, v_t, y_t, 1.0 / D, EPS)

            if kv_mode:
                # evacuate psum via ACT so gpsimd can do the rope muls
                psb = sml.tile([128, 512], F32, tag="psb", bufs=1)
                nc.scalar.copy(psb, pp)
                src = psb
                mul_eng = nc.gpsimd
            else:
                src = pp
                mul_eng = nc.vector
            pr = src.rearrange("p (h d2 two) -> p h d2 two", h=HC, two=2)
            pe = pr[:, :, :, 0]
            po = pr[:, :, :, 1]

            def hb(col):
                sl = tab[:, col:col + 32]
                return ap_with(sl, [sl.ap[0], [0, HC], sl.ap[1]])
            cqe, sqo, cqo, sqe = hb(0), hb(32), hb(64), hb(96)
            m1 = sml.tile([128, HC, 32], F32, tag="m1", bufs=2)
            m2 = sml.tile([128, HC, 32], F32, tag="m2", bufs=2)
            m3 = sml.tile([128, HC, 32], F32, tag="m3", bufs=2)
            m4 = sml.tile([128, HC, 32], F32, tag="m4", bufs=2)
            mul_eng.tensor_mul(m1, pe, cqe)
            mul_eng.tensor_mul(m2, po, sqo)
            mul_eng.tensor_mul(m3, po, cqo)
            mul_eng.tensor_mul(m4, pe, sqe)
            pre = sml.tile([128, HC, 2, 32], F32, tag="pre", bufs=2)
            nc.vector.tensor_sub(pre[:, :, 0, :], m1, m2)
            nc.vector.tensor_add(pre[:, :, 1, :], m3, m4)
            rope = sml.tile([128, 512], F32R, tag="rope", bufs=2)
            rr_b = ap_with(rr, [rr.ap[0], rr.ap[1], [0, D]])
            nc.vector.tensor_mul(rope.rearrange("p (h d) -> p h d", h=HC),
                                 pre.rearrange("p h a b -> p h (a b)"), rr_b)
            if defer is not None:
                return (rope, dstT_col, kv_mode)
            emit_transposes(rope, dstT_col, kv_mode)

        def emit_transposes(rope, dstT_col, kv_mode):
            for s in range(ST_):
                tp = psB.tile([128, 128], F32R, tag="mix", bufs=1)
                nc.tensor.transpose(tp, rope[:, 128 * s:128 * (s + 1)], ident)
                if kv_mode and s % 2 == 0:
                    nc.scalar.copy(dstT_col(s), tp)
                else:
                    nc.vector.tensor_copy(dstT_col(s), tp)

        def _body():
            # ===== prelude: stage disjoint input slices, AllGather on device
            if cc:
                # internal DRAM: collective bounce + gathered operands
                # (allocated per rep from a bufs=2 pool -> double-buffered;
                # only the 8-core AG output may be pair-Shared HBM).
                # x is gathered in two chunks so the KV loop can start after
                # the first; prelude order = wwo (gates first matmul), x_a,
                # tables (needed at the first K-norm), x_b (needed at nt=4).
                x_bnc_a = dram.tile([128, 4, CT, 128], BF16, tag="x_bnc_a")
                x_bnc_b = dram.tile([128, 4, CT, 128], BF16, tag="x_bnc_b")
                xg_a = dram.tile([2, 128, 4, CT, 128], BF16, tag="xg_a")
                xg_b = dram.tile([2, 128, 4, CT, 128], BF16, tag="xg_b")
                wwo_bnc = dram.tile([128, 2 * 1536 + C], BF16, tag="wwo_bnc")
                wwog = dram.tile([4, 128, 2 * 1536 + C], BF16, tag="wwog")
                t_bnc = dram.tile([512, 128], F32, tag="t_bnc")
                tg = dram.tile([8, 512, 128], F32, tag="tg",
                               addr_space="Shared")
                y_bnc = dram.tile([N, C], BF16, tag="y_bnc")
                yr_a = dram.tile([N // 4, C], BF16, tag="yr_a")
                yr_b = dram.tile([N // 4, C], BF16, tag="yr_b")
                nc.sync.dma_start(wwo_bnc[:], wwo_d)
                nc.sync.dma_start(x_bnc_a[:], xh_d[:, 0:4, :, :])
                nc.sync.dma_start(x_bnc_b[:], xh_d[:, 4:8, :, :])
                nc.sync.dma_start(t_bnc[:], t_d)
                nc.gpsimd.collective_compute(
                    "AllGather", ALU.bypass, replica_groups=QUADS,
                    ins=[wwo_bnc.opt()], outs=[wwog.opt()])
                nc.gpsimd.collective_compute(
                    "AllGather", ALU.bypass, replica_groups=PAIRS,
                    ins=[x_bnc_a.opt()], outs=[xg_a.opt()])
                nc.gpsimd.collective_compute(
                    "AllGather", ALU.bypass, replica_groups=OCT,
                    ins=[t_bnc.opt()], outs=[tg.opt()])
                nc.gpsimd.collective_compute(
                    "AllGather", ALU.bypass, replica_groups=PAIRS,
                    ins=[x_bnc_b.opt()], outs=[xg_b.opt()])
                w_src = lambda r, j: wwog[r, :, 1536 * j:1536 * (j + 1)]
                wo_src = lambda s: wwog[s, :, 3072:3072 + C]

                def x_src(nt):
                    r, l = divmod(nt, 8)
                    return (xg_a[r, :, l, :, :] if l < 4
                            else xg_b[r, :, l - 4, :, :])
            else:
                tg, y_bnc = tg_d, y_d
                w_src = lambda r, j: wg_d[r, :, j, :]
                wo_src = lambda s: wog_d[s, :, 0, :]

                def x_src(nt):
                    return xg_d[nt // 8, :, nt % 8, :, :]
            for t in range(CT):
                r, j = divmod(t, 2)
                nc.sync.dma_start(w_all[:, t, :], w_src(r, j))

            def t_tile(dst, nt, is_k):
                off = 256 * int(is_k) + 128 * (nt % 2)
                nc.sync.dma_start(dst, tg[nt // 2, off:off + 128, :])

            def x_tile(dst, nt):
                nc.sync.dma_start(dst, x_src(nt))

            # ================= Phase KV =================
            pending_tp = None
            for nt in range(NT):
                    n0 = 128 * nt
                    nsl = 0
                    xt = ph1.tile([128, CT, 128], BF16, tag="xt", bufs=3)
                    x_tile(xt, nt)
                    tk_sb = sml.tile([128, 128], F32, tag="tk")
                    t_tile(tk_sb, nt, is_k=True)
                    vp = psA.tile([128, 1024], F32, tag="st", name="vp")[:, 0:512]
                    qkv_matmuls(vp, xt, nsl, w_all, 1024)
                    nc.scalar.copy(Vg[:, nt, :, 0:D],
                                   vp.rearrange("p (h d) -> p h d", h=HC))
                    kp = psA.tile([128, 1024], F32, tag="st", name="kp")[:, 0:512]
                    qkv_matmuls(kp, xt, nsl, w_all, 512)
                    if pending_tp is not None:
                        emit_transposes(*pending_tp)
                    pending_tp = norm_rope_transpose(
                        kp, tk_sb, (lambda n0=n0: (lambda s: KT[:, s, n0:n0 + 128]))(),
                        defer=True)

            if pending_tp is not None:
                emit_transposes(*pending_tp)
            for s in range(ST_):
                nc.sync.dma_start(wo_sb[:, s, :], wo_src(s))

            def q_subtile(qt_tile, ci, j, act_evac=False):
                """Q for n-subtile j (of 4) of q-block ci -> qt_tile[:, s, 128j:]."""
                nt = 4 * ci + j
                xtq = ph1.tile([128, CT, 128], BF16, tag="xt", name="xtq", bufs=3)
                x_tile(xtq, nt)
                tq_sb = sml.tile([128, 128], F32, tag="tk")
                t_tile(tq_sb, nt, is_k=False)
                qp = psA.tile([128, 512], F32, tag="qk", bufs=1)
                qkv_matmuls(qp, xtq, 0, w_all, 0)
                norm_rope_transpose(
                    qp, tq_sb, lambda s: qt_tile[:, s, 128 * j:128 * (j + 1)],
                    kv_mode=act_evac)

            def proj_tile(ot_tile, ci, ntl, cc, alt=False):
                """alt=True (tail only, attention done): use a psA 'st' bank
                so back-to-back projs don't serialize on the single 'mix'
                buffer's DVE evacuation."""
                nt = 4 * ci + ntl
                if alt:
                    yp = psA.tile([128, 1024], F32, tag="st",
                                  name="yp_alt")[:, 0:512]
                else:
                    yp = psB.tile([128, 512], F32, tag="mix", bufs=1)
                for s in range(ST_):
                    nc.tensor.matmul(yp, ot_tile[:, s, 128 * ntl:128 * (ntl + 1)],
                                     wo_sb[:, s, 512 * cc:512 * (cc + 1)],
                                     start=(s == 0), stop=(s == ST_ - 1))
                ysb = ph3.tile([128, 512], BF16, tag="ysb")
                nc.vector.tensor_copy(ysb, yp)
                nc.sync.dma_start(
                    y_bnc[128 * nt:128 * (nt + 1), 512 * cc:512 * (cc + 1)], ysb)

            # ================= per q-block: attn (+ next Q, prev proj) ==========
            QT = qt_p.tile([128, ST_, 512], F32R, tag="QT")
            for j in range(4):
                q_subtile(QT, 0, j, act_evac=True)
            prev = None  # (OT, ci) pending projection

            for ci in range(4):
                OT = ot_p.tile([128, ST_, 512], BF16, tag="OT")
                QT_next = None
                if ci + 1 < 4:
                    QT_next = qt_p.tile([128, ST_, 512], F32R, tag="QT")
                for hp in range(ST_):
                    u = psB.tile([D + 1, 1024], F32, tag="u", bufs=1)
                    es = []
                    for kt in range(KT_):
                        st = psA.tile([128, 1024], F32, tag="st")
                        nc.tensor.matmul(st[:, 0:512],
                                         KT[0:64, hp, 128 * kt:128 * (kt + 1)],
                                         QT[0:64, hp, :],
                                         start=True, stop=True, tile_position=(0, 0))
                        nc.tensor.matmul(st[:, 512:1024],
                                         KT[64:128, hp, 128 * kt:128 * (kt + 1)],
                                         QT[64:128, hp, :],
                                         start=True, stop=True, tile_position=(64, 0))
                        e = ph2.tile([128, 1024], F32R, tag="E", bufs=2)
                        if kt in DVE_EXP_KT:
                            emit_dve_exp(nc, e, st, SCALE)
                        else:
                            nc.scalar.activation(e, st, AF.Exp, scale=SCALE)
                        es.append((kt, e))
                        if len(es) > 2:
                            pk, pe_ = es.pop(0)
                            nc.tensor.matmul(u[:, 0:512], Vg[:, pk, 2 * hp, :],
                                             pe_[:, 0:512],
                                             start=(pk == 0), stop=False)
                            nc.tensor.matmul(u[:, 512:1024], Vg[:, pk, 2 * hp + 1, :],
                                             pe_[:, 512:1024],
                                             start=(pk == 0), stop=False)
                    while es:
                        pk, pe_ = es.pop(0)
                        nc.tensor.matmul(u[:, 0:512], Vg[:, pk, 2 * hp, :],
                                         pe_[:, 0:512],
                                         start=(pk == 0), stop=(pk == KT_ - 1))
                        nc.tensor.matmul(u[:, 512:1024], Vg[:, pk, 2 * hp + 1, :],
                                         pe_[:, 512:1024],
                                         start=(pk == 0), stop=(pk == KT_ - 1))

                    # evacuate U fast to free the PSUM bank, normalize off-path
                    usb = ph2.tile([D + 1, 1024], F32, tag="usb", bufs=1)
                    nc.vector.tensor_copy(usb, u)
                    den = ph2.tile([1, 1024], F32, tag="den", bufs=1)
                    nc.vector.tensor_copy(den, usb[D:D + 1, :])
                    rcp = ph2.tile([1, 1024], F32, tag="rcp", bufs=1)
                    nc.vector.reciprocal_approx_fast(rcp, den)
                    bc = ph2.tile([64, 1024], F32, tag="bc", bufs=1)
                    nc.gpsimd.partition_broadcast(bc, rcp)
                    for e_i in range(2):
                        nc.vector.tensor_mul(
                            OT[64 * e_i:64 * (e_i + 1), hp, :],
                            usb[0:D, 512 * e_i:512 * (e_i + 1)],
                            bc[:, 512 * e_i:512 * (e_i + 1)])

                    # interleave: one Q subtile of next block + 2 proj tiles of prev
                    if QT_next is not None:
                        q_subtile(QT_next, ci + 1, hp)
                    if prev is not None:
                        proj_tile(prev[0], prev[1], hp, 0)
                        proj_tile(prev[0], prev[1], hp, 1)

                prev = (OT, ci)
                QT = QT_next
                if cc and ci == 2:
                    # rows 0:N/2 of y_bnc are complete (ci=0 projs ran during
                    # ci=1, ci=1 projs during ci=2) -> overlap first RS with
                    # the remaining attention compute
                    nc.gpsimd.collective_compute(
                        "ReduceScatter", ALU.add, replica_groups=PAIRS,
                        ins=[y_bnc[0:N // 2, :].opt()], outs=[yr_a.opt()])

            for ntl in range(4):
                proj_tile(prev[0], prev[1], ntl, 0, alt=True)
                proj_tile(prev[0], prev[1], ntl, 1)

            # ===== tail: pair-sum the remaining partial output rows
            if cc:
                nc.gpsimd.collective_compute(
                    "ReduceScatter", ALU.add, replica_groups=PAIRS,
                    ins=[y_bnc[N // 2:N, :].opt()], outs=[yr_b.opt()])
                nc.sync.dma_start(y_d[0:N // 4, :], yr_a[:])
                nc.sync.dma_start(y_d[N // 4:N // 2, :], yr_b[:])

        for _rep in range(rep):
            _body()


    nc.compile()
    return nc


def make_tables(freqs_cos, freqs_sin, nw):
    """Host: fold norm weight into rope tables. [N, 128] f32:
    cols 0:32=cqe, 32:64=sqo, 64:96=cqo, 96:128=sqe."""
    cos_p = np.asarray(freqs_cos)[:, 0::2]
    sin_p = np.asarray(freqs_sin)[:, 0::2]
    nw = np.asarray(nw)
    ne = nw[0::2][None, :]
    no = nw[1::2][None, :]
    return np.concatenate([cos_p * ne, sin_p * no, cos_p * no, sin_p * ne],
                          axis=1).astype(np.float32)


def shard_inputs(x, w_qkv, w_proj, b_proj, qn_w, kn_w, freqs_cos, freqs_sin):
    """Returns in_maps for 8 cores. Core c: batch c//2, head group c%2.

    Each core gets only a DISJOINT bf16 slice; the kernel AllGathers:
      xh   [128, 8, CT, 128]  x^T for n-half (c%2) of batch c//2
      wqkv [128, 2, 1536]     ct-quarter (c//2) of head-group (c%2) cols
      wo   [128, 1, C]        row-stack (c//2) of head-group (c%2)
      tqk  [512, 128]         rows 256c..256c+255 of [tq; tk] (f32)
    """
    import ml_dtypes
    BF = ml_dtypes.bfloat16
    x = np.asarray(x); w_qkv = np.asarray(w_qkv); w_proj = np.asarray(w_proj)
    tq_t = make_tables(freqs_cos, freqs_sin, qn_w).reshape(8, 2, 128, 128)
    tk_t = make_tables(freqs_cos, freqs_sin, kn_w).reshape(8, 2, 128, 128)
    tqk = np.concatenate([tq_t, tk_t], axis=1).reshape(8, 512, 128)

    xT_b = []
    for b in range(B):
        xb = x[b].astype(BF).reshape(NT, 128, CT, 128).transpose(3, 0, 2, 1)
        xT_b.append(xb)
    w_bf = w_qkv.astype(BF)
    wg_l = []
    for g in range(2):
        cols = slice(512 * g, 512 * (g + 1))
        wq_g = np.concatenate(
            [w_bf[:, 0:C][:, cols], w_bf[:, C:2 * C][:, cols],
             w_bf[:, 2 * C:3 * C][:, cols]], axis=1)
        wg_l.append(wq_g.reshape(CT, 128, 3 * 512))
    wo_bf = w_proj.astype(BF)

    in_maps = []
    for c in range(8):
        b, g = c // 2, c % 2
        wq_in = wg_l[g][2 * b:2 * b + 2].transpose(1, 0, 2).reshape(128, 3072)
        wo_in = wo_bf[512 * g:512 * (g + 1)].reshape(ST_, 128, C)[b]
        in_maps.append({
            "xh": np.ascontiguousarray(xT_b[b][:, 8 * g:8 * (g + 1)]),
            "wwo": np.ascontiguousarray(
                np.concatenate([wq_in, wo_in], axis=1)),
            "tqk": np.ascontiguousarray(tqk[c]),
        })
    return in_maps


def gather_outputs(results, b_proj):
    """Core 2b: y[0:512] = rows 0:512, y[512:1024] = rows 1024:1536 (RS rank
    0 shard of each half-RS); core 2b+1: rank-1 shards."""
    out = np.empty((B, N, C), dtype=np.float32)
    bp = np.asarray(b_proj, dtype=np.float32)
    Q = N // 4
    for b in range(B):
        y0 = results[2 * b]["y"].astype(np.float32)
        y1 = results[2 * b + 1]["y"].astype(np.float32)
        out[b, 0 * Q:1 * Q] = y0[0:Q] + bp
        out[b, 1 * Q:2 * Q] = y1[0:Q] + bp
        out[b, 2 * Q:3 * Q] = y0[Q:2 * Q] + bp
        out[b, 3 * Q:4 * Q] = y1[Q:2 * Q] + bp
    return out


_CACHED = {}


def _make_runner(nc, n_cores=8):
    """Build the jitted SPMD dispatch once (same mechanism as
    run_bass_kernel_spmd's axon path, but cached across calls, with
    donated output buffers zero-filled ON DEVICE instead of shipped from
    host). Returns (dispatch, zero_fns, in_names, out_names, out_avals,
    sharding)."""
    import jax
    from jax.sharding import Mesh, PartitionSpec, NamedSharding
    from jax.experimental.shard_map import shard_map
    from concourse import bass2jax

    bass2jax.install_neuronx_cc_hook()
    partition_name = (nc.partition_id_tensor.name
                      if nc.partition_id_tensor else None)
    in_names, out_names, out_avals = [], [], []
    for alloc in nc.m.functions[0].allocations:
        if not isinstance(alloc, mybir.MemoryLocationSet):
            continue
        name = alloc.memorylocations[0].name
        if alloc.kind == "ExternalInput":
            if name != partition_name:
                in_names.append(name)
        elif alloc.kind == "ExternalOutput":
            out_names.append(name)
            out_avals.append(jax.core.ShapedArray(
                tuple(alloc.tensor_shape), mybir.dt.np(alloc.dtype)))
    n_params, n_outs = len(in_names), len(out_avals)
    all_in = in_names + out_names + ([partition_name] if partition_name else [])

    def _body(*args):
        operands = list(args)
        if partition_name:
            operands.append(bass2jax.partition_id_tensor())
        return tuple(bass2jax._bass_exec_p.bind(
            *operands, out_avals=tuple(out_avals), in_names=tuple(all_in),
            out_names=tuple(out_names), lowering_input_output_aliases=(),
            sim_require_finite=True, sim_require_nnan=True, nc=nc))

    donate = tuple(range(n_params, n_params + n_outs))
    mesh = Mesh(np.asarray(jax.devices()[:n_cores]), ("core",))
    spec = NamedSharding(mesh, PartitionSpec("core"))
    in_specs = (PartitionSpec("core"),) * (n_params + n_outs)
    out_specs = (PartitionSpec("core"),) * n_outs
    dispatch = jax.jit(
        shard_map(_body, mesh=mesh, in_specs=in_specs, out_specs=out_specs,
                  check_rep=False),
        donate_argnums=donate, keep_unused=True)
    zero_fns = [jax.jit(
        (lambda s, d: (lambda: jax.numpy.zeros((n_cores * s[0],) + s[1:], d)))(
            tuple(a.shape), a.dtype),
        out_shardings=spec) for a in out_avals]
    return dispatch, zero_fns, in_names, out_names, out_avals, spec


def _run(runner, in_maps):
    dispatch, zero_fns, in_names, out_names, out_avals, _ = runner
    n = len(in_maps)
    concat_in = [np.concatenate([np.asarray(in_maps[c][nm]) for c in range(n)],
                                axis=0) for nm in in_names]
    zeros = [f() for f in zero_fns]
    outs = dispatch(*concat_in, *zeros)
    outs_np = [np.asarray(a) for a in outs]
    return [{nm: outs_np[i].reshape(n, *out_avals[i].shape)[c]
             for i, nm in enumerate(out_names)} for c in range(n)]


def kernel(x, w_qkv, w_proj, b_proj, qn_w, kn_w, freqs_cos, freqs_sin):
    """Full-input entry point; shards across 8 NeuronCores, returns [B,N,C]."""
    in_maps = shard_inputs(x, w_qkv, w_proj, b_proj, qn_w, kn_w,
                           freqs_cos, freqs_sin)
    if "nc" not in _CACHED:
        _CACHED["nc"] = build_core_kernel(num_devices=8)
    nc = _CACHED["nc"]
    try:
        if "runner" not in _CACHED:
            _CACHED["runner"] = _make_runner(nc, 8)
        res = _run(_CACHED["runner"], in_maps)
    except Exception:
        res = run_bass_kernel_spmd(nc, in_maps, core_ids=list(range(8))).results
    return gather_outputs(res, b_proj)

